# revision 37
# baseline (speedup 1.0000x reference)
"""AttentionSynapse kernel for Trainium2 (8 NeuronCores, SPMD).

reference math:
    wq_n = wq / ||wq||_e ; wk_n = wk / ||wq||_e          (both normed by wq's norm)
    q = gq @ wq_n ; k = gk @ wk_n                        [b,h,t,z]
    a = (q @ k^T) / sqrt(z), diag -> -inf
    out = logsumexp(a, axis=-1)                          [b,h,t]

kernel math (per core: one batch b, 4 heads):
    s_z   = 1 / (8 * sum_e wq[h,e,z]^2)                  (fold norm^2 and 1/sqrt(64) into k)
    qT    = (gq @ wq)^T          [z,t]  (bf16, 2 heads packed into 128 partitions)
    kTs   = (gk @ wk)^T * s_z    [z,t]
    S     = qT^T @ kTs           [t,s]  per 128-row t-tile, f32 in PSUM
    sums[t]  = sum_s exp(S[t,s])       (ACT exp with fused accum)
    diag[t]  = S[t,t]                  (DVE mul-by-identity + reduce)
    out[t]   = ln(sums[t] - exp(diag[t]))

dataflow per core:
    weights: one HWDGE load -> squares/sums (ACT/DVE) -> n2 via PE ones-matmul
    g: SWDGE cast fp32->bf16 DRAM->SBUF (8 quarter-chunks, gk first)
       -> PE 128x128 transposes (PSUM) -> DVE evacuate -> gT [e,t] SBUF
    projections: PE, 2 heads packed into M=128; kT scaled by s_z on PSUM evac
    scores: row-tiled K=64 matmul pairs -> [128,2048] f32 PSUM (2 bufs)
    exp+row-sum: single ACT instruction per tile (Exp with fused accum_out);
    diag via DVE identity-mask mul + reduce; finalize ln(sums-exp(diag))
    timing: ACT is the floor (64 tiles x ~2.25us); prep ~95us; ~255us e2e
"""

import numpy as np

T = 2048
E = 1024
Z = 64
HLOC = 4  # heads per core
NCORES = 8

_CACHE = {}


def _build():
    """Build + compile the per-core Bass program (same program on all cores)."""
    from contextlib import ExitStack

    import concourse.bass as bass
    import concourse.mybir as mybir
    import concourse.tile as tile
    from concourse import bacc

    fp32 = mybir.dt.float32
    bf16 = mybir.dt.bfloat16
    AF = mybir.ActivationFunctionType
    ALU = mybir.AluOpType

    nc = bacc.Bacc(
        "TRN2",
        target_bir_lowering=False,
        debug=False,
        enable_asserts=False,
        num_devices=NCORES,
    )

    # inputs: g[0]=gk, g[1]=gq (gk first: kTs is on the critical path)
    # w[0]=wq, w[1]=wk, packed [e, 4*64] with col = hl*64 + z (hl = local head)
    g = nc.dram_tensor("g", [2, T, E], fp32, kind="ExternalInput").ap()
    w = nc.dram_tensor("w", [2, E, HLOC * Z], fp32, kind="ExternalInput").ap()
    # output stored [hl, p, i] with t = i*128 + p; host transposes on unshard
    o = nc.dram_tensor("o", [HLOC, 128, T // 128], fp32, kind="ExternalOutput").ap()

    with tile.TileContext(nc) as tc, ExitStack() as ctx:
        persist = ctx.enter_context(tc.tile_pool(name="persist", bufs=1))
        dram = ctx.enter_context(tc.tile_pool(name="dram", bufs=1, space="DRAM"))

        from concourse.masks import make_identity

        ident_sb = persist.tile([128, 128], fp32, tag="ident", name="ident_sb")
        make_identity(nc, ident_sb[:])
        eights = persist.tile([128, 1], fp32, tag="eights", name="eights")
        nc.gpsimd.memset(eights[:], 8.0)

        # ---------------- weights: ONE load (keeps HW lane ticks minimal), ----
        # ---------------- square-sum (wq), cast to bf16 ----------------------
        w_bf = {}
        s_col = {}
        with tc.tile_pool(name="psum_a", bufs=2, space="PSUM") as psum_a:
            wall = persist.tile([128, 16 * 256], fp32, tag="wall", name="wall")
            # [p, (wt, a, z)] <- w[wt, a*128+p, z]
            nc.sync.dma_start(
                wall.rearrange("p (wt a z) -> p wt a z", wt=2, a=8),
                w.rearrange("wt (a p) z -> p wt a z", p=128),
            )
            wf = {}
            for wt in range(2):
                for a in range(8):
                    wf[wt, a] = wall[:, (wt * 8 + a) * 256 : (wt * 8 + a + 1) * 256]
            ssq = persist.tile([128, 256], fp32, tag="ssq", name="ssq")
            # deprioritize weight processing in the modeled schedule so the
            # g casts (critical path) get minimal wait ticks
            with tc.tile_wait_until(0.012):
                for wt in range(2):
                    for a in range(8):
                        wb = persist.tile(
                            [128, 256], bf16, tag=f"wb{wt}_{a}", name=f"wb{wt}_{a}"
                        )
                        nc.vector.tensor_copy(wb[:], wf[wt, a])
                        w_bf[wt, a] = wb
                        if wt == 0:
                            if a == 0:
                                nc.scalar.activation(ssq[:], wf[wt, a], AF.Square)
                            else:
                                sq = persist.tile(
                                    [128, 256], fp32, tag=f"sq{a}", name=f"sq{a}"
                                )
                                nc.scalar.activation(sq[:], wf[wt, a], AF.Square)
                                nc.vector.tensor_add(ssq[:], ssq[:], sq[:])
            # n2col[p] = 8 * sum_e wq[e, pg*128+p]^2   (via matmul with 8.0-vector)
            for pg in range(2):
                n2p = psum_a.tile([128, 1], fp32, tag="n2p", name="n2p")
                nc.tensor.matmul(
                    n2p[:],
                    ssq[:, pg * 128 : (pg + 1) * 128],
                    eights[:],
                    start=True,
                    stop=True,
                )
                s_sb = persist.tile([128, 1], fp32, tag=f"scol{pg}", name=f"scol{pg}")
                nc.vector.reciprocal(s_sb[:], n2p[:])
                s_col[pg] = s_sb

        # ---------------- G: cast->bf16 (DRAM), transpose-DMA, project -------
        qT = {}
        kTs = {}
        for pg in range(2):
            qT[pg] = persist.tile([128, T], bf16, tag=f"qT{pg}", name=f"qT{pg}")
            kTs[pg] = persist.tile([128, T], bf16, tag=f"kTs{pg}", name=f"kTs{pg}")
        gT = {}
        for gi in range(2):
            for a in range(8):
                gT[gi, a] = persist.tile(
                    [128, T], bf16, tag=f"gT{gi}_{a}", name=f"gT{gi}_{a}"
                )

        with (
            tc.tile_pool(name="headp", bufs=2) as head_pool,
            tc.tile_pool(name="prodp", bufs=2) as prod_pool,
        ):
            prep_ctx = ExitStack()
            psum_p = prep_ctx.enter_context(
                tc.tile_pool(name="psum_p", bufs=2, space="PSUM")
            )
            psum_t = prep_ctx.enter_context(
                tc.tile_pool(name="psum_t", bufs=6, space="PSUM")
            )
            gsbp = prep_ctx.enter_context(tc.tile_pool(name="gsbp", bufs=8))
            ident_bf = persist.tile([128, 128], bf16, tag="identbf", name="ident_bf")
            nc.vector.tensor_copy(ident_bf[:], ident_sb[:])

            def quarter(gi, qr):
                """cast + PE-transpose + projections for one 512-row quarter."""
                wt = 1 - gi
                r0 = qr * 512
                gsb = gsbp.tile([128, 4 * E], bf16, tag="gsb", name="gsb")
                nc.gpsimd.dma_start(
                    gsb.rearrange("p (st e) -> p st e", st=4),
                    g[gi, r0 : r0 + 512, :].rearrange("(st p) e -> p st e", p=128),
                )
                for st in range(4):
                    tb = r0 + st * 128
                    for a in range(8):
                        pt = psum_t.tile([128, 128], bf16, tag="pt", name="pt")
                        nc.tensor.transpose(
                            pt[:],
                            gsb[:, st * E + a * 128 : st * E + (a + 1) * 128],
                            ident_bf[:],
                        )
                        nc.vector.tensor_copy(gT[gi, a][:, tb : tb + 128], pt[:])
                for pg in range(2):
                    acc = psum_p.tile([128, 512], fp32, tag="acc", name="acc")
                    for a in range(8):
                        nc.tensor.matmul(
                            acc[:],
                            w_bf[wt, a][:, pg * 128 : (pg + 1) * 128],
                            gT[gi, a][:, r0 : r0 + 512],
                            start=(a == 0),
                            stop=(a == 7),
                        )
                    dst = (kTs if gi == 0 else qT)[pg][:, r0 : r0 + 512]
                    if gi == 0:
                        nc.vector.tensor_scalar_mul(dst, acc[:], s_col[pg][:])
                    else:
                        nc.vector.tensor_copy(dst, acc[:])

            sums_t = {}
            diag_t = {}
            for hl in range(HLOC):
                sums_t[hl] = head_pool.tile(
                    [128, 16], fp32, tag=f"sums{hl}", name=f"sums{hl}"
                )
                diag_t[hl] = head_pool.tile(
                    [128, 16], fp32, tag=f"diagT{hl}", name=f"diagT{hl}"
                )

            def score_tile(psum_s, pg, hh, i):
                hl = pg * 2 + hh
                sc = psum_s.tile([128, T], fp32, tag="sc", name="sc")
                for sq4 in range(4):
                    nc.tensor.matmul(
                        sc[:, sq4 * 512 : (sq4 + 1) * 512],
                        qT[pg][hh * 64 : (hh + 1) * 64, i * 128 : (i + 1) * 128],
                        kTs[pg][hh * 64 : (hh + 1) * 64, sq4 * 512 : (sq4 + 1) * 512],
                        start=True,
                        stop=True,
                        tile_position=(hh * 64, 0),
                    )
                prod = prod_pool.tile([128, 128], fp32, tag="prod", name="prod")
                nc.vector.tensor_mul(
                    prod[:], sc[:, i * 128 : (i + 1) * 128], ident_sb[:]
                )
                nc.vector.tensor_reduce(
                    diag_t[hl][:, i : i + 1],
                    prod[:],
                    axis=mybir.AxisListType.X,
                    op=ALU.add,
                )
                # exp in-place in PSUM: out is scratch, and a PSUM dest has
                # lower ScalarE fixed overhead than an SBUF dest
                nc.scalar.activation(
                    sc[:], sc[:], AF.Exp, accum_out=sums_t[hl][:, i : i + 1]
                )

            # gk first (kTs is needed by every score tile), then gq
            for gi in range(2):
                for qr in range(4):
                    quarter(gi, qr)
            prep_ctx.close()
            with tc.tile_pool(name="psum_s", bufs=2, space="PSUM") as psum_s:
                for pg in range(2):
                    for hh in range(2):
                        for i in range(16):
                            score_tile(psum_s, pg, hh, i)

            # ------ finalize all heads at the end: ln(sums - exp(diag)) ------
            for hl in range(HLOC):
                expd = head_pool.tile(
                    [128, 16], fp32, tag=f"expd{hl}", name=f"expd{hl}"
                )
                nc.scalar.activation(expd[:], diag_t[hl][:], AF.Exp)
                corr = head_pool.tile(
                    [128, 16], fp32, tag=f"corr{hl}", name=f"corr{hl}"
                )
                nc.vector.tensor_sub(corr[:], sums_t[hl][:], expd[:])
                logt = head_pool.tile(
                    [128, 16], fp32, tag=f"logt{hl}", name=f"logt{hl}"
                )
                nc.scalar.activation(logt[:], corr[:], AF.Ln)
                nc.sync.dma_start(o[hl], logt[:])

    nc.compile()
    return nc


def _get_nc():
    if "nc" not in _CACHE:
        _CACHE["nc"] = _build()
    return _CACHE["nc"]


def make_in_maps(gq, gk, wq, wk):
    """Host-side sharding: core i -> batch i//4, heads 4*(i%4) .. +4."""
    in_maps = []
    for core in range(NCORES):
        b = core // 4
        h0 = HLOC * (core % 4)
        # pack heads into columns: [e, hl*64+z]
        wq_c = np.ascontiguousarray(
            wq[h0 : h0 + HLOC].transpose(1, 0, 2).reshape(E, HLOC * Z)
        )
        wk_c = np.ascontiguousarray(
            wk[h0 : h0 + HLOC].transpose(1, 0, 2).reshape(E, HLOC * Z)
        )
        in_maps.append(
            {
                "g": np.ascontiguousarray(np.stack([gk[b], gq[b]])),
                "w": np.ascontiguousarray(np.stack([wq_c, wk_c])),
            }
        )
    return in_maps


def kernel(gq, gk, wq, wk, _trace=False):
    from concourse import bass_utils

    nc = _get_nc()
    in_maps = make_in_maps(gq, gk, wq, wk)
    res = bass_utils.run_bass_kernel_spmd(
        nc, in_maps, core_ids=list(range(NCORES)), trace=_trace
    )
    if _trace:
        _CACHE["last_results"] = res
    b_h = gq.shape[0]
    h = wq.shape[0]
    out = np.empty((b_h, h, T), dtype=np.float32)
    for core in range(NCORES):
        b = core // 4
        h0 = HLOC * (core % 4)
        oc = res.results[core]["o"]  # [HLOC, 128, 16], t = i*128 + p
        for hl in range(HLOC):
            out[b, h0 + hl, :] = oc[hl].T.ravel()
    return out
